# revision 2
# baseline (speedup 1.0000x reference)
import sys

sys.path.insert(0, "/opt/trn_rl_repo")
import numpy as np
import concourse.bass as bass
import concourse.bacc as bacc
import concourse.mybir as mybir
import concourse.tile as tile
from concourse import masks
import concourse.bass_utils as bass_utils

bass_utils.upload_artifacts = lambda tmpdir: "local://" + tmpdir
from concourse.bass_utils import run_bass_kernel_spmd

N_CORES = 8
B, H, W, C, R = 32, 56, 56, 256, 16
BS = B // N_CORES          # 4 samples per core
NP = H * W                 # 3136 pixels per sample
NT = 25                    # tiles per sample: 24 x 128 + 1 x 64
ROWS = BS * NP             # 12544 rows per core
F32 = mybir.dt.float32
AL = mybir.AluOpType
AF = mybir.ActivationFunctionType
AX = mybir.AxisListType

_COMPILED = None


def _build():
    nc = bacc.Bacc(None, target_bir_lowering=False, num_devices=N_CORES)
    x_d = nc.declare_dram_parameter("x", [ROWS, C], F32, isOutput=False)
    w1_d = nc.declare_dram_parameter("w1", [C, R], F32, isOutput=False)
    b1_d = nc.declare_dram_parameter("b1", [1, R], F32, isOutput=False)
    w2_d = nc.declare_dram_parameter("w2", [R, C], F32, isOutput=False)
    b2_d = nc.declare_dram_parameter("b2", [1, C], F32, isOutput=False)
    wf_d = nc.declare_dram_parameter("wflat", [98, 1], F32, isOutput=False)
    bc_d = nc.declare_dram_parameter("bconv", [1, 1], F32, isOutput=False)
    out_d = nc.declare_dram_parameter("out", [ROWS, C], F32, isOutput=True)

    flat_dram = nc.dram_tensor("flat_dram", [2 * BS, NP], F32)
    fpad_dram = nc.dram_tensor("fpad_dram", [2 * BS, 3844], F32)

    with tile.TileContext(nc) as tc:
        with tc.tile_pool(name="const", bufs=1) as cp, \
             tc.tile_pool(name="xbuf", bufs=1) as xp, \
             tc.tile_pool(name="work", bufs=3) as wp, \
             tc.tile_pool(name="sp", bufs=3) as spp, \
             tc.tile_pool(name="psA", bufs=2, space="PSUM") as psA, \
             tc.tile_pool(name="psB", bufs=3, space="PSUM") as psB, \
             tc.tile_pool(name="psC", bufs=3, space="PSUM") as psC:

            # ---------- constants ----------
            ident = cp.tile([128, 128], F32)
            masks.make_identity(nc, ident[:])
            ones2 = cp.tile([2, 128], F32)
            nc.gpsimd.memset(ones2[:], 1.0)

            w1t = cp.tile([128, 2 * R], F32)       # [K-chunk, 2*16]
            nc.sync.dma_start(w1t[:, 0:R], w1_d[0:128, :])
            nc.sync.dma_start(w1t[:, R:2 * R], w1_d[128:256, :])
            w2t = cp.tile([R, C], F32)
            nc.sync.dma_start(w2t[:], w2_d[:])
            wf_t = cp.tile([98, 1], F32)
            nc.sync.dma_start(wf_t[:], wf_d[:])

            b1r = cp.tile([1, R], F32)
            nc.sync.dma_start(b1r[:], b1_d[:])
            b1b = cp.tile([2, R], F32)
            nc.gpsimd.partition_broadcast(b1b[:], b1r[:], channels=2)
            b2r = cp.tile([1, C], F32)
            nc.sync.dma_start(b2r[:], b2_d[:])
            b2b = cp.tile([2, C], F32)
            nc.gpsimd.partition_broadcast(b2b[:], b2r[:], channels=2)
            bcr = cp.tile([1, 1], F32)
            nc.sync.dma_start(bcr[:], bc_d[:])
            bcb = cp.tile([128, 1], F32)
            nc.gpsimd.partition_broadcast(bcb[:], bcr[:], channels=128)

            # zero the padded-plane dram scratch (borders stay zero forever)
            zrow = cp.tile([2 * BS, 3844], F32)
            nc.vector.memset(zrow[:], 0.0)
            nc.sync.dma_start(fpad_dram.ap(), zrow[:])

            # resident x (overwritten in place by xg then by out)
            xbuf = xp.tile([128, BS * NT * C], F32)

            def xt(s, t):
                pt = 64 if t == NT - 1 else 128
                return xbuf[0:pt, (s * NT + t) * C:(s * NT + t + 1) * C]

            def xrows(s, t):
                r0 = s * NP + t * 128
                pt = 64 if t == NT - 1 else 128
                return x_d[r0:r0 + pt, :], out_d[r0:r0 + pt, :]

            maxacc_l, rhs_l, cb_l, spm_l, spx_l, spsc_l = {}, {}, {}, {}, {}, {}

            for s in range(BS):
                for t in range(NT):
                    src, _ = xrows(s, t)
                    nc.sync.dma_start(xt(s, t), src)

            for s in range(BS):
                # ============ phase A: load + pooling stats ============
                acc0 = psA.tile([128, 128], F32, tag="acc")
                acc1 = psA.tile([128, 128], F32, tag="acc")
                accs = [acc0, acc1]
                maxacc = wp.tile([128, C], F32, tag="maxacc")
                for t in range(NT):
                    pt = 64 if t == NT - 1 else 128
                    v = xt(s, t)
                    for c in range(2):
                        nc.tensor.matmul(
                            accs[c][:, 0:pt], v[:, c * 128:(c + 1) * 128],
                            ident[0:pt, 0:pt], is_transpose=True,
                            start=(t == 0), stop=(t == NT - 1),
                            skip_group_check=True)
                    if t == 0:
                        nc.vector.tensor_copy(maxacc[:], v)
                    else:
                        nc.vector.tensor_tensor(
                            out=maxacc[0:pt, :], in0=maxacc[0:pt, :], in1=v,
                            op=AL.max)

                # ============ phase A finalize: pooled vectors ============
                rhs_s = wp.tile([128, 4], F32, tag="rhs")
                for c in range(2):
                    tmp = wp.tile([128, 1], F32, tag="redtmp")
                    nc.vector.reduce_sum(tmp[:], accs[c][:], axis=AX.X)
                    nc.scalar.activation(rhs_s[:, 2 * c:2 * c + 1], tmp[:],
                                         AF.Copy, scale=1.0 / NP)
                    mt = psB.tile([128, 128], F32, tag="psb")
                    nc.tensor.transpose(mt[:], maxacc[:, c * 128:(c + 1) * 128],
                                        ident[:])
                    nc.vector.reduce_max(rhs_s[:, 2 * c + 1:2 * c + 2], mt[:],
                                         axis=AX.X)

                # ============ phase B: MLP -> channel scale row ============
                h_ps = psB.tile([2, R], F32, tag="psb")
                nc.tensor.matmul(h_ps[:], rhs_s[:, 0:2], w1t[:, 0:R],
                                 start=True, stop=False)
                nc.tensor.matmul(h_ps[:], rhs_s[:, 2:4], w1t[:, R:2 * R],
                                 start=False, stop=True)
                hb = wp.tile([2, R], F32, tag="hb")
                nc.vector.tensor_tensor(out=hb[:], in0=h_ps[:], in1=b1b[:],
                                        op=AL.add)
                hr = wp.tile([2, R], F32, tag="hr")
                nc.scalar.activation(hr[:], hb[:], AF.Relu)
                hT_ps = psB.tile([R, 2], F32, tag="psb")
                nc.tensor.transpose(hT_ps[:], hr[:], ident[0:2, 0:2])
                hT = wp.tile([R, 2], F32, tag="hT")
                nc.vector.tensor_copy(hT[:], hT_ps[:])
                co_ps = psB.tile([2, C], F32, tag="psb")
                nc.tensor.matmul(co_ps[:], hT[:], w2t[:], start=True, stop=True)
                co_sb = wp.tile([2, C], F32, tag="co")
                nc.vector.tensor_tensor(out=co_sb[:], in0=co_ps[:], in1=b2b[:],
                                        op=AL.add)
                sig = wp.tile([2, C], F32, tag="sig")
                nc.scalar.activation(sig[:], co_sb[:], AF.Sigmoid)
                cb_ps = psB.tile([128, C], F32, tag="psb")
                nc.tensor.matmul(cb_ps[:], ones2[:], sig[:], start=True, stop=True)
                cb = wp.tile([128, C], F32, tag="cb")
                nc.vector.tensor_copy(cb[:], cb_ps[:])

                # ============ phase C: xg (in place) + spatial stats ============
                spm = spp.tile([128, NT], F32, tag="spm")
                spx = spp.tile([128, NT], F32, tag="spx")
                nc.vector.memset(spm[64:128, NT - 1:NT], 0.0)
                nc.vector.memset(spx[64:128, NT - 1:NT], 0.0)
                for t in range(NT):
                    pt = 64 if t == NT - 1 else 128
                    v = xt(s, t)
                    nc.vector.tensor_tensor(out=v, in0=v, in1=cb[0:pt, :],
                                            op=AL.mult)
                    nc.vector.reduce_max(spx[0:pt, t:t + 1], v, axis=AX.X)
                    nc.scalar.activation(v, v, AF.Copy,
                                         accum_out=spm[0:pt, t:t + 1])

                # ============ phase D: 7x7x2 conv via patch matmuls ============
                for c, sp_t in enumerate((spm, spx)):
                    row = s * 2 + c
                    spT_ps = psB.tile([NT, 128], F32, tag="psb")
                    nc.tensor.transpose(spT_ps[:], sp_t[:], ident[:])
                    spT = wp.tile([NT, 128], F32, tag="spT")
                    nc.vector.tensor_copy(spT[:], spT_ps[:])
                    nc.sync.dma_start(
                        bass.AP(flat_dram, row * NP, [[128, 24], [1, 128]]),
                        spT[0:24, :])
                    nc.sync.dma_start(
                        bass.AP(flat_dram, row * NP + 3072, [[1, 64]]),
                        spT[24:25, 0:64])
                    nc.sync.dma_start(
                        bass.AP(fpad_dram, row * 3844 + 3 * 62 + 3,
                                [[62, 56], [1, 56]]),
                        bass.AP(flat_dram, row * NP, [[56, 56], [1, 56]]))
                patches = wp.tile([98, NP], F32, tag="patches")
                for c in range(2):
                    row = s * 2 + c
                    for dy in range(7):
                        nc.sync.dma_start(
                            patches[c * 49 + dy * 7:c * 49 + dy * 7 + 7, :],
                            bass.AP(fpad_dram, row * 3844 + dy * 62,
                                    [[1, 7], [62, 56], [1, 56]]))
                conv_ps = psC.tile([128, NT], F32, tag="conv")
                for t in range(NT):
                    pt = 64 if t == NT - 1 else 128
                    nc.tensor.matmul(conv_ps[0:pt, t:t + 1],
                                     patches[:, t * 128:t * 128 + pt],
                                     wf_t[:], start=True, stop=True,
                                     skip_group_check=True)
                nc.vector.memset(conv_ps[64:128, NT - 1:NT], 0.0)
                spsc = spp.tile([128, NT], F32, tag="spsc")
                nc.scalar.activation(spsc[:], conv_ps[:], AF.Sigmoid, bias=bcb[:])

                # ============ phase E: out = xg * spatial, store ============
                for t in range(NT):
                    pt = 64 if t == NT - 1 else 128
                    v = xt(s, t)
                    _, dst = xrows(s, t)
                    nc.scalar.activation(v, v, AF.Copy,
                                         scale=spsc[0:pt, t:t + 1])
                    nc.sync.dma_start(dst, v)

    nc.compile()
    return nc


def _get_compiled():
    global _COMPILED
    if _COMPILED is None:
        _COMPILED = _build()
    return _COMPILED


def _make_in_maps(inputs):
    x = np.ascontiguousarray(np.asarray(inputs["x"], dtype=np.float32))
    # wconv [7,7,2,1] -> wflat[k] = wconv[dy,dx,c]; k = c*49 + dy*7 + dx
    wf = np.asarray(inputs["wconv"], dtype=np.float32)[:, :, :, 0]
    wf = wf.transpose(2, 0, 1).copy()
    wf[0] /= C          # fold the channel-mean (1/256) into the conv weight
    wf = wf.reshape(98, 1)
    xs = x.reshape(N_CORES, ROWS, C)
    return [{
        "x": xs[i],
        "w1": np.asarray(inputs["w1"], np.float32),
        "b1": np.asarray(inputs["b1"], np.float32).reshape(1, R),
        "w2": np.asarray(inputs["w2"], np.float32),
        "b2": np.asarray(inputs["b2"], np.float32).reshape(1, C),
        "wflat": wf,
        "bconv": np.asarray(inputs["bconv"], np.float32).reshape(1, 1),
    } for i in range(N_CORES)]


def kernel(x, w1, b1, w2, b2, wconv, bconv):
    nc = _get_compiled()
    in_maps = _make_in_maps(dict(x=x, w1=w1, b1=b1, w2=w2, b2=b2,
                                 wconv=wconv, bconv=bconv))
    res = run_bass_kernel_spmd(nc, in_maps, list(range(N_CORES)))
    out = np.stack([res.results[i]["out"] for i in range(N_CORES)], axis=0)
    return out.reshape(B, H, W, C)



# revision 7
# speedup vs baseline: 1.7731x; 1.7731x over previous
import sys

sys.path.insert(0, "/opt/trn_rl_repo")
import numpy as np
import ml_dtypes
import concourse.bass as bass
import concourse.bacc as bacc
import concourse.mybir as mybir
import concourse.tile as tile
from concourse import masks
import concourse.bass_utils as bass_utils

bass_utils.upload_artifacts = lambda tmpdir: "local://" + tmpdir
from concourse.bass_utils import run_bass_kernel_spmd

N_CORES = 8
B, H, W, C, R = 32, 56, 56, 256, 16
BS = B // N_CORES          # 4 samples per core
NP = H * W                 # 3136 valid pixels per sample
PW = 64                    # padded plane width (8 zero cols; x-wrap is SAME pad)
NTP = 32                   # tiles in 64x64 padded plane
VT0, VT1 = 1, 29           # plane tiles holding data (inclusive)
NVT = VT1 - VT0 + 1        # 29 tiles per sample
PP = 112                   # packed partitions: 2 rows x 56 valid cols
CW = NVT * C               # 7424 sbuf cols per sample
BF16 = mybir.dt.bfloat16
F32 = mybir.dt.float32
AL = mybir.AluOpType
AF = mybir.ActivationFunctionType
AX = mybir.AxisListType

_COMPILED = None


def _build():
    nc = bacc.Bacc(None, target_bir_lowering=False, num_devices=N_CORES)
    x_d = nc.declare_dram_parameter("x", [PP, BS * CW], BF16, isOutput=False)
    w1_d = nc.declare_dram_parameter("w1", [C, R], F32, isOutput=False)
    b1_d = nc.declare_dram_parameter("b1", [1, R], F32, isOutput=False)
    w2_d = nc.declare_dram_parameter("w2", [R, C], F32, isOutput=False)
    b2_d = nc.declare_dram_parameter("b2", [1, C], F32, isOutput=False)
    wb_d = nc.declare_dram_parameter("wband", [PP, 10 * PP], F32, isOutput=False)
    bc_d = nc.declare_dram_parameter("bconv", [1, 1], F32, isOutput=False)
    out_d = nc.declare_dram_parameter("out", [PP, BS * CW], BF16, isOutput=True)

    with tile.TileContext(nc) as tc:
        with tc.tile_pool(name="const", bufs=1) as cp, \
             tc.tile_pool(name="xbuf", bufs=1) as xp, \
             tc.tile_pool(name="work", bufs=3) as wp, \
             tc.tile_pool(name="sp", bufs=2) as spp, \
             tc.tile_pool(name="psPool", bufs=2, space="PSUM") as psP, \
             tc.tile_pool(name="psMlp", bufs=1, space="PSUM") as psM, \
             tc.tile_pool(name="psB", bufs=2, space="PSUM") as psC:

            # ---------- constants ----------
            identb = cp.tile([PP, PP], BF16)
            masks.make_identity(nc, identb[:])
            ident2 = cp.tile([2, 2], F32)
            masks.make_identity(nc, ident2[:])
            onesb = cp.tile([PP, 1], BF16)
            nc.gpsimd.memset(onesb[:], 1.0)
            ones2 = cp.tile([2, PP], BF16)
            nc.gpsimd.memset(ones2[:], 1.0)

            w1t = cp.tile([128, 2 * R], F32)
            nc.sync.dma_start(w1t[:, 0:R], w1_d[0:128, :])
            nc.sync.dma_start(w1t[:, R:2 * R], w1_d[128:256, :])
            w2t = cp.tile([R, C], F32)
            nc.sync.dma_start(w2t[:], w2_d[:])
            wb = cp.tile([PP, 10 * PP], F32)
            nc.sync.dma_start(wb[:], wb_d[:])

            b1r = cp.tile([1, R], F32)
            nc.sync.dma_start(b1r[:], b1_d[:])
            b1b = cp.tile([2, R], F32)
            nc.gpsimd.partition_broadcast(b1b[:], b1r[:], channels=2)
            b2r = cp.tile([1, C], F32)
            nc.sync.dma_start(b2r[:], b2_d[:])
            b2b = cp.tile([2, C], F32)
            nc.gpsimd.partition_broadcast(b2b[:], b2r[:], channels=2)
            bcr = cp.tile([1, 1], F32)
            nc.sync.dma_start(bcr[:], bc_d[:])
            bcb = cp.tile([PP, 1], F32)
            nc.gpsimd.partition_broadcast(bcb[:], bcr[:], channels=PP)

            # resident x in packed padded-plane layout (overwritten in place)
            xbuf = xp.tile([PP, BS * CW], BF16)

            def xt(s, t):
                return xbuf[:, (s * NVT + t) * C:(s * NVT + t + 1) * C]

            for s in range(BS):
                nc.sync.dma_start(xbuf[:, s * CW:(s + 1) * CW],
                                  x_d[:, s * CW:(s + 1) * CW])

            for s in range(BS):
                # ===== phase A: channel pooling (sum via PE, max via DVE) ====
                accs = [psP.tile([128, 1], F32, tag="poolacc",
                                 name=f"acc{s}_{i}") for i in range(2)]
                maxs = [wp.tile([PP, C], BF16, tag=f"maxacc{i}",
                                name=f"maxs{s}_{i}") for i in range(2)]
                for t in range(NVT):
                    v = xt(s, t)
                    for c in range(2):
                        nc.tensor.matmul(accs[c][:], v[:, c * 128:(c + 1) * 128],
                                         onesb[:], start=(t == 0),
                                         stop=(t == NVT - 1),
                                         skip_group_check=True)
                    i = t % 2
                    if t < 2:
                        nc.vector.tensor_copy(maxs[i][:], v)
                    else:
                        nc.vector.tensor_tensor(out=maxs[i][:], in0=maxs[i][:],
                                                in1=v, op=AL.max)
                nc.vector.tensor_tensor(out=maxs[0][:], in0=maxs[0][:],
                                        in1=maxs[1][:], op=AL.max)
                maxacc = maxs[0]

                # pooled vectors -> rhs_s [128, 4]: (avg0, max0, avg1, max1)
                rhs_s = wp.tile([128, 4], F32, tag="rhs")
                for c in range(2):
                    nc.scalar.activation(rhs_s[:, 2 * c:2 * c + 1], accs[c][:],
                                         AF.Copy, scale=1.0 / NP)
                    mt = psC.tile([128, PP], BF16, tag="psmT",
                                  name=f"mt{s}_{c}")
                    nc.tensor.transpose(mt[:], maxacc[:, c * 128:(c + 1) * 128],
                                        identb[:])
                    nc.vector.reduce_max(rhs_s[:, 2 * c + 1:2 * c + 2], mt[:],
                                         axis=AX.X)

                # ===== phase B: MLP -> per-channel scale (sum of 2 sigmoids) ==
                mlp_ps = psM.tile([128, 512], F32, tag="psb")
                h_ps = mlp_ps[0:2, 0:R]
                hT_ps = mlp_ps[0:R, R:R + 2]
                co_ps = mlp_ps[0:2, 256:512]
                nc.tensor.matmul(h_ps, rhs_s[:, 0:2], w1t[:, 0:R],
                                 start=True, stop=False, skip_group_check=True)
                nc.tensor.matmul(h_ps, rhs_s[:, 2:4], w1t[:, R:2 * R],
                                 start=False, stop=True, skip_group_check=True)
                hb = wp.tile([2, R], F32, tag="hb")
                nc.vector.tensor_tensor(out=hb[:], in0=h_ps, in1=b1b[:],
                                        op=AL.add)
                hr = wp.tile([2, R], F32, tag="hr")
                nc.scalar.activation(hr[:], hb[:], AF.Relu)
                nc.tensor.transpose(hT_ps, hr[:], ident2[:])
                hT = wp.tile([R, 2], F32, tag="hT")
                nc.vector.tensor_copy(hT[:], hT_ps)
                nc.tensor.matmul(co_ps, hT[:], w2t[:], start=True, stop=True,
                                 skip_group_check=True)
                co_sb = wp.tile([2, C], F32, tag="co")
                nc.vector.tensor_tensor(out=co_sb[:], in0=co_ps, in1=b2b[:],
                                        op=AL.add)
                sig = wp.tile([2, C], BF16, tag="sig")
                nc.scalar.activation(sig[:], co_sb[:], AF.Sigmoid)
                cb_ps = psM.tile([PP, C], F32, tag="pscb")
                nc.tensor.matmul(cb_ps[:], ones2[:], sig[:], start=True,
                                 stop=True, skip_group_check=True)
                cb = wp.tile([PP, C], BF16, tag="cb")
                nc.vector.tensor_copy(cb[:], cb_ps[:])

                # ===== phase C: xg (in place) + spatial mean/max ============
                spm = spp.tile([PP, 36], F32, tag="spm")
                spx = spp.tile([PP, 36], F32, tag="spx")
                nc.vector.memset(spm[:], 0.0)
                nc.vector.memset(spx[:], 0.0)
                for t in range(NVT):
                    v = xt(s, t)
                    nc.vector.tensor_tensor(out=v, in0=v, in1=cb[:], op=AL.mult)
                    nc.vector.reduce_max(spx[:, t + 3:t + 4], v, axis=AX.X)
                    nc.scalar.activation(v, v, AF.Copy,
                                         accum_out=spm[:, t + 3:t + 4])

                # ===== phase D: 7x7x2 conv as 10 banded matmuls =============
                conv_ps = psC.tile([PP, NVT], F32, tag="conv",
                                   name=f"conv{s}")
                k = 0
                for ch, sp_t in enumerate((spm, spx)):
                    for j in range(-2, 3):
                        nc.tensor.matmul(
                            conv_ps[:],
                            wb[:, (ch * 5 + j + 2) * PP:(ch * 5 + j + 3) * PP],
                            sp_t[:, 3 + j:32 + j],
                            start=(k == 0), stop=(k == 9),
                            skip_group_check=True)
                        k += 1
                spsc = wp.tile([PP, NVT], F32, tag="spsc")
                nc.scalar.activation(spsc[:], conv_ps[:], AF.Sigmoid,
                                     bias=bcb[:])

                # ===== phase E: out = xg * spatial (in place), store ========
                for t in range(NVT):
                    v = xt(s, t)
                    nc.scalar.activation(v, v, AF.Copy, scale=spsc[:, t:t + 1])
                nc.scalar.dma_start(out_d[:, s * CW:(s + 1) * CW],
                                    xbuf[:, s * CW:(s + 1) * CW])

    nc.compile()
    return nc


def _get_compiled():
    global _COMPILED
    if _COMPILED is None:
        _COMPILED = _build()
    return _COMPILED


def _band_matrices(wconv):
    # Wb[ch, j+2, i, o]: contribution of packed input row i (tile tau+j) to
    # packed output row o (tile tau). Packed row i <-> padded row
    # i + 8*(i>=56); plane offset (dy-3)*PW + (dx-3) in flat 64x64 layout.
    Wb = np.zeros((2, 5, PP, PP), np.float32)
    for ch in range(2):
        wch = wconv[:, :, ch, 0].astype(np.float32)
        if ch == 0:
            wch = wch / C      # fold channel-mean 1/C into the conv weight
        for dy in range(7):
            for dx in range(7):
                off = (dy - 3) * PW + (dx - 3)
                for o in range(PP):
                    r = o + 8 * (o >= 56)        # padded output row
                    t = r + off
                    j = t // 128
                    rpp = t - j * 128            # padded input row
                    if rpp < 56:
                        i = rpp
                    elif 64 <= rpp < 120:
                        i = rpp - 8
                    else:
                        continue                 # lands in a zero pad row
                    Wb[ch, j + 2, i, o] = wch[dy, dx]
    return np.ascontiguousarray(Wb.transpose(2, 0, 1, 3).reshape(PP, 10 * PP))


def _pack_x(x):
    # [B,H,W,C] f32 -> per-core [PP, BS*CW] bf16 in packed padded-plane layout
    xc = np.asarray(x, np.float32).reshape(N_CORES, BS, H, W, C)
    P = np.zeros((N_CORES, BS, PW, PW, C), np.float32)
    P[:, :, 3:59, 0:56, :] = xc
    T = P.reshape(N_CORES, BS, NTP, 128, C)[:, :, VT0:VT1 + 1]
    Tp = np.concatenate([T[:, :, :, 0:56, :], T[:, :, :, 64:120, :]], axis=3)
    Tp = Tp.transpose(0, 3, 1, 2, 4)             # [NC, PP, BS, NVT, C]
    return np.ascontiguousarray(
        Tp.reshape(N_CORES, PP, BS * CW)).astype(ml_dtypes.bfloat16)


def _unpack_out(res):
    # per-core [PP, BS*CW] bf16 -> [B,H,W,C] f32
    o = np.stack([np.asarray(res[i], np.float32) for i in range(N_CORES)], 0)
    o = o.reshape(N_CORES, PP, BS, NVT, C).transpose(0, 2, 3, 1, 4)
    O = np.zeros((N_CORES, BS, NTP, 128, C), np.float32)
    O[:, :, VT0:VT1 + 1, 0:56] = o[:, :, :, 0:56]
    O[:, :, VT0:VT1 + 1, 64:120] = o[:, :, :, 56:112]
    return O.reshape(N_CORES, BS, PW, PW, C)[:, :, 3:59, 0:56, :].reshape(
        B, H, W, C)


def _make_in_maps(inputs):
    xp = _pack_x(inputs["x"])
    wband = _band_matrices(np.asarray(inputs["wconv"], np.float32))
    w1 = np.asarray(inputs["w1"], np.float32)
    b1 = np.asarray(inputs["b1"], np.float32).reshape(1, R)
    w2 = np.asarray(inputs["w2"], np.float32)
    b2 = np.asarray(inputs["b2"], np.float32).reshape(1, C)
    bc = np.asarray(inputs["bconv"], np.float32).reshape(1, 1)
    return [{
        "x": xp[i], "w1": w1, "b1": b1, "w2": w2, "b2": b2,
        "wband": wband, "bconv": bc,
    } for i in range(N_CORES)]


def kernel(x, w1, b1, w2, b2, wconv, bconv):
    nc = _get_compiled()
    in_maps = _make_in_maps(dict(x=x, w1=w1, b1=b1, w2=w2, b2=b2,
                                 wconv=wconv, bconv=bconv))
    res = run_bass_kernel_spmd(nc, in_maps, list(range(N_CORES)))
    return _unpack_out([res.results[i]["out"] for i in range(N_CORES)])


# revision 13
# speedup vs baseline: 2.1539x; 1.2148x over previous
import sys

sys.path.insert(0, "/opt/trn_rl_repo")
import numpy as np
import ml_dtypes
import concourse.bass as bass
import concourse.bacc as bacc
import concourse.mybir as mybir
import concourse.tile as tile
from concourse import masks
import concourse.bass_utils as bass_utils

bass_utils.upload_artifacts = lambda tmpdir: "local://" + tmpdir
from concourse.bass_utils import run_bass_kernel_spmd

N_CORES = 8
B, H, W, C, R = 32, 56, 56, 256, 16
BS = B // N_CORES          # 4 samples per core
NP = H * W                 # 3136 valid pixels per sample
PW = 64                    # padded plane width (8 zero cols; x-wrap is SAME pad)
NTP = 32                   # tiles in 64x64 padded plane
VT0, VT1 = 1, 29           # plane tiles holding data (inclusive)
NVT = VT1 - VT0 + 1        # 29 tiles per sample
PP = 112                   # packed partitions: 2 rows x 56 valid cols
CW = NVT * C               # 7424 sbuf cols per sample
BF16 = mybir.dt.bfloat16
F32 = mybir.dt.float32
AL = mybir.AluOpType
AF = mybir.ActivationFunctionType
AX = mybir.AxisListType

_COMPILED = None


def _build():
    nc = bacc.Bacc(None, target_bir_lowering=False, num_devices=N_CORES)
    x_d = nc.declare_dram_parameter("x", [PP, BS * CW], BF16, isOutput=False)
    w1_d = nc.declare_dram_parameter("w1", [C, R], F32, isOutput=False)
    b1_d = nc.declare_dram_parameter("b1", [1, R], F32, isOutput=False)
    w2_d = nc.declare_dram_parameter("w2", [R, C], F32, isOutput=False)
    b2_d = nc.declare_dram_parameter("b2", [1, C], F32, isOutput=False)
    wb_d = nc.declare_dram_parameter("wband", [PP, 10 * PP], BF16,
                                     isOutput=False)
    bc_d = nc.declare_dram_parameter("bconv", [1, 1], F32, isOutput=False)
    out_d = nc.declare_dram_parameter("out", [PP, BS * CW], BF16, isOutput=True)

    with tile.TileContext(nc) as tc:
        with tc.tile_pool(name="const", bufs=1) as cp, \
             tc.tile_pool(name="xbuf", bufs=1) as xp, \
             tc.tile_pool(name="work", bufs=3) as wp, \
             tc.tile_pool(name="sp", bufs=2) as spp, \
             tc.tile_pool(name="psPool", bufs=2, space="PSUM") as psP, \
             tc.tile_pool(name="psMlp", bufs=1, space="PSUM") as psM, \
             tc.tile_pool(name="psB", bufs=2, space="PSUM") as psC:

            # ---------- constants ----------
            identb = cp.tile([PP, PP], BF16)
            masks.make_identity(nc, identb[:])
            ident2 = cp.tile([2, 2], F32)
            masks.make_identity(nc, ident2[:])
            onesb = cp.tile([PP, 1], BF16)
            nc.gpsimd.memset(onesb[:], 1.0)
            ones2 = cp.tile([2, PP], BF16)
            nc.gpsimd.memset(ones2[:], 1.0)

            w1t = cp.tile([128, 2 * R], F32)
            nc.sync.dma_start(w1t[:, 0:R], w1_d[0:128, :])
            nc.sync.dma_start(w1t[:, R:2 * R], w1_d[128:256, :])
            w2t = cp.tile([R, C], F32)
            nc.sync.dma_start(w2t[:], w2_d[:])
            wb = cp.tile([PP, 10 * PP], BF16)
            nc.sync.dma_start(wb[:], wb_d[:])

            b1r = cp.tile([1, R], F32)
            nc.sync.dma_start(b1r[:], b1_d[:])
            b1b = cp.tile([2, R], F32)
            nc.gpsimd.partition_broadcast(b1b[:], b1r[:], channels=2)
            b2r = cp.tile([1, C], F32)
            nc.sync.dma_start(b2r[:], b2_d[:])
            b2b = cp.tile([2, C], F32)
            nc.gpsimd.partition_broadcast(b2b[:], b2r[:], channels=2)
            bcr = cp.tile([1, 1], F32)
            nc.sync.dma_start(bcr[:], bc_d[:])
            bcb = cp.tile([PP, 1], F32)
            nc.gpsimd.partition_broadcast(bcb[:], bcr[:], channels=PP)

            # resident x in packed padded-plane layout (overwritten in place)
            xbuf = xp.tile([PP, BS * CW], BF16)

            def xt(s, t):
                return xbuf[:, (s * NVT + t) * C:(s * NVT + t + 1) * C]

            HCW = (NVT // 2) * C           # first 14 tiles
            for s in range(BS):
                c0 = s * CW
                nc.sync.dma_start(xbuf[:, c0:c0 + HCW], x_d[:, c0:c0 + HCW])
                nc.sync.dma_start(xbuf[:, c0 + HCW:c0 + CW],
                                  x_d[:, c0 + HCW:c0 + CW])

            for s in range(BS):
                # ===== phase A: channel pooling (sum via PE, max via DVE) ====
                accs = [psP.tile([128, 1], F32, tag="poolacc",
                                 name=f"acc{s}_{i}") for i in range(2)]
                maxs = [wp.tile([PP, C], BF16, tag=f"maxacc{i}",
                                name=f"maxs{s}_{i}") for i in range(2)]
                for t in range(NVT):
                    v = xt(s, t)
                    for c in range(2):
                        nc.tensor.matmul(accs[c][:], v[:, c * 128:(c + 1) * 128],
                                         onesb[:], start=(t == 0),
                                         stop=(t == NVT - 1),
                                         skip_group_check=True)
                    i = t % 2
                    if t < 2:
                        nc.vector.tensor_copy(maxs[i][:], v)
                    else:
                        nc.vector.tensor_tensor(out=maxs[i][:], in0=maxs[i][:],
                                                in1=v, op=AL.max)
                nc.vector.tensor_tensor(out=maxs[0][:], in0=maxs[0][:],
                                        in1=maxs[1][:], op=AL.max)
                maxacc = maxs[0]

                # pooled vectors -> rhs_s [128, 4]: (avg0, max0, avg1, max1)
                rhs_s = wp.tile([128, 4], F32, tag="rhs")
                for c in range(2):
                    nc.scalar.activation(rhs_s[:, 2 * c:2 * c + 1], accs[c][:],
                                         AF.Copy, scale=1.0 / NP)
                    mt = psC.tile([128, PP], BF16, tag="psmT",
                                  name=f"mt{s}_{c}")
                    nc.tensor.transpose(mt[:], maxacc[:, c * 128:(c + 1) * 128],
                                        identb[:])
                    nc.vector.reduce_max(rhs_s[:, 2 * c + 1:2 * c + 2], mt[:],
                                         axis=AX.X)

                # ===== phase B: MLP -> per-channel scale (sum of 2 sigmoids) ==
                mlp_ps = psM.tile([128, 512], F32, tag="psb")
                h_ps = mlp_ps[0:2, 0:R]
                hT_ps = mlp_ps[0:R, R:R + 2]
                co_ps = mlp_ps[0:2, 256:512]
                nc.tensor.matmul(h_ps, rhs_s[:, 0:2], w1t[:, 0:R],
                                 start=True, stop=False, skip_group_check=True)
                nc.tensor.matmul(h_ps, rhs_s[:, 2:4], w1t[:, R:2 * R],
                                 start=False, stop=True, skip_group_check=True)
                hb = wp.tile([2, R], F32, tag="hb")
                nc.vector.tensor_tensor(out=hb[:], in0=h_ps, in1=b1b[:],
                                        op=AL.add)
                hr = wp.tile([2, R], F32, tag="hr")
                nc.scalar.activation(hr[:], hb[:], AF.Relu)
                nc.tensor.transpose(hT_ps, hr[:], ident2[:])
                hT = wp.tile([R, 2], F32, tag="hT")
                nc.vector.tensor_copy(hT[:], hT_ps)
                nc.tensor.matmul(co_ps, hT[:], w2t[:], start=True, stop=True,
                                 skip_group_check=True)
                co_sb = wp.tile([2, C], F32, tag="co")
                nc.vector.tensor_tensor(out=co_sb[:], in0=co_ps, in1=b2b[:],
                                        op=AL.add)
                sig = wp.tile([2, C], BF16, tag="sig")
                nc.scalar.activation(sig[:], co_sb[:], AF.Sigmoid)
                cb_ps = psM.tile([PP, C], F32, tag="pscb")
                nc.tensor.matmul(cb_ps[:], ones2[:], sig[:], start=True,
                                 stop=True, skip_group_check=True)
                cb = wp.tile([PP, C], BF16, tag="cb")
                nc.vector.tensor_copy(cb[:], cb_ps[:])

                # ===== phase C: xg (in place) + spatial mean/max ============
                # single sample-wide DVE ops on 3D access patterns
                v3 = xbuf[:, s * CW:(s + 1) * CW].rearrange(
                    "p (t c) -> p t c", c=C)
                cb3 = cb[:].unsqueeze(1).broadcast_to([PP, NVT, C])
                nc.vector.tensor_tensor(out=v3, in0=v3, in1=cb3, op=AL.mult)
                spm_f = wp.tile([PP, NVT], F32, tag="spmf")
                nc.vector.reduce_sum(spm_f[:], v3, axis=AX.X)
                spm = spp.tile([PP, 36], BF16, tag="spm")
                spx = spp.tile([PP, 36], BF16, tag="spx")
                nc.vector.memset(spm[:], 0.0)
                nc.vector.memset(spx[:], 0.0)
                nc.vector.tensor_copy(spm[:, 3:32], spm_f[:])
                nc.vector.reduce_max(spx[:, 3:32], v3, axis=AX.X)

                # ===== phase D: 7x7x2 conv as 10 banded matmuls =============
                conv_ps = psC.tile([PP, NVT], F32, tag="conv",
                                   name=f"conv{s}")
                k = 0
                for ch, sp_t in enumerate((spm, spx)):
                    for j in range(-2, 3):
                        nc.tensor.matmul(
                            conv_ps[:],
                            wb[:, (ch * 5 + j + 2) * PP:(ch * 5 + j + 3) * PP],
                            sp_t[:, 3 + j:32 + j],
                            start=(k == 0), stop=(k == 9),
                            skip_group_check=True)
                        k += 1
                spsc = wp.tile([PP, NVT], F32, tag="spsc")
                nc.scalar.activation(spsc[:], conv_ps[:], AF.Sigmoid,
                                     bias=bcb[:])

                # ===== phase E: out = xg * spatial (in place), store ========
                for t in range(NVT):
                    v = xt(s, t)
                    nc.scalar.activation(v, v, AF.Copy, scale=spsc[:, t:t + 1])
                    if t == NVT // 2 - 1:
                        nc.scalar.dma_start(
                            out_d[:, s * CW:s * CW + HCW],
                            xbuf[:, s * CW:s * CW + HCW])
                nc.scalar.dma_start(out_d[:, s * CW + HCW:(s + 1) * CW],
                                    xbuf[:, s * CW + HCW:(s + 1) * CW])

    nc.compile()
    return nc


def _get_compiled():
    global _COMPILED
    if _COMPILED is None:
        _COMPILED = _build()
    return _COMPILED


def _band_matrices(wconv):
    # Wb[ch, j+2, i, o]: contribution of packed input row i (tile tau+j) to
    # packed output row o (tile tau). Packed row i <-> padded row
    # i + 8*(i>=56); plane offset (dy-3)*PW + (dx-3) in flat 64x64 layout.
    Wb = np.zeros((2, 5, PP, PP), np.float32)
    for ch in range(2):
        wch = wconv[:, :, ch, 0].astype(np.float32)
        if ch == 0:
            wch = wch / C      # fold channel-mean 1/C into the conv weight
        for dy in range(7):
            for dx in range(7):
                off = (dy - 3) * PW + (dx - 3)
                for o in range(PP):
                    r = o + 8 * (o >= 56)        # padded output row
                    t = r + off
                    j = t // 128
                    rpp = t - j * 128            # padded input row
                    if rpp < 56:
                        i = rpp
                    elif 64 <= rpp < 120:
                        i = rpp - 8
                    else:
                        continue                 # lands in a zero pad row
                    Wb[ch, j + 2, i, o] = wch[dy, dx]
    return np.ascontiguousarray(
        Wb.transpose(2, 0, 1, 3).reshape(PP, 10 * PP)).astype(
            ml_dtypes.bfloat16)


def _pack_x(x):
    # [B,H,W,C] f32 -> per-core [PP, BS*CW] bf16 in packed padded-plane layout
    xc = np.asarray(x, np.float32).reshape(N_CORES, BS, H, W, C)
    P = np.zeros((N_CORES, BS, PW, PW, C), np.float32)
    P[:, :, 3:59, 0:56, :] = xc
    T = P.reshape(N_CORES, BS, NTP, 128, C)[:, :, VT0:VT1 + 1]
    Tp = np.concatenate([T[:, :, :, 0:56, :], T[:, :, :, 64:120, :]], axis=3)
    Tp = Tp.transpose(0, 3, 1, 2, 4)             # [NC, PP, BS, NVT, C]
    return np.ascontiguousarray(
        Tp.reshape(N_CORES, PP, BS * CW)).astype(ml_dtypes.bfloat16)


def _unpack_out(res):
    # per-core [PP, BS*CW] bf16 -> [B,H,W,C] f32
    o = np.stack([np.asarray(res[i], np.float32) for i in range(N_CORES)], 0)
    o = o.reshape(N_CORES, PP, BS, NVT, C).transpose(0, 2, 3, 1, 4)
    O = np.zeros((N_CORES, BS, NTP, 128, C), np.float32)
    O[:, :, VT0:VT1 + 1, 0:56] = o[:, :, :, 0:56]
    O[:, :, VT0:VT1 + 1, 64:120] = o[:, :, :, 56:112]
    return O.reshape(N_CORES, BS, PW, PW, C)[:, :, 3:59, 0:56, :].reshape(
        B, H, W, C)


def _make_in_maps(inputs):
    xp = _pack_x(inputs["x"])
    wband = _band_matrices(np.asarray(inputs["wconv"], np.float32))
    w1 = np.asarray(inputs["w1"], np.float32)
    b1 = np.asarray(inputs["b1"], np.float32).reshape(1, R)
    w2 = np.asarray(inputs["w2"], np.float32)
    b2 = np.asarray(inputs["b2"], np.float32).reshape(1, C)
    bc = np.asarray(inputs["bconv"], np.float32).reshape(1, 1)
    return [{
        "x": xp[i], "w1": w1, "b1": b1, "w2": w2, "b2": b2,
        "wband": wband, "bconv": bc,
    } for i in range(N_CORES)]


def kernel(x, w1, b1, w2, b2, wconv, bconv):
    nc = _get_compiled()
    in_maps = _make_in_maps(dict(x=x, w1=w1, b1=b1, w2=w2, b2=b2,
                                 wconv=wconv, bconv=bconv))
    res = run_bass_kernel_spmd(nc, in_maps, list(range(N_CORES)))
    return _unpack_out([res.results[i]["out"] for i in range(N_CORES)])


# revision 15
# speedup vs baseline: 2.1591x; 1.0024x over previous
import sys

sys.path.insert(0, "/opt/trn_rl_repo")
import numpy as np
import ml_dtypes
import concourse.bass as bass
import concourse.bacc as bacc
import concourse.mybir as mybir
import concourse.tile as tile
from concourse import masks
import concourse.bass_utils as bass_utils

bass_utils.upload_artifacts = lambda tmpdir: "local://" + tmpdir
from concourse.bass_utils import run_bass_kernel_spmd

N_CORES = 8
B, H, W, C, R = 32, 56, 56, 256, 16
BS = B // N_CORES          # 4 samples per core
NP = H * W                 # 3136 valid pixels per sample
PW = 64                    # padded plane width (8 zero cols; x-wrap is SAME pad)
NTP = 32                   # tiles in 64x64 padded plane
VT0, VT1 = 1, 29           # plane tiles holding data (inclusive)
NVT = VT1 - VT0 + 1        # 29 tiles per sample
PP = 112                   # packed partitions: 2 rows x 56 valid cols
CW = NVT * C               # 7424 sbuf cols per sample
BF16 = mybir.dt.bfloat16
F32 = mybir.dt.float32
AL = mybir.AluOpType
AF = mybir.ActivationFunctionType
AX = mybir.AxisListType

_COMPILED = None


def _build():
    nc = bacc.Bacc(None, target_bir_lowering=False, num_devices=N_CORES)
    x_d = nc.declare_dram_parameter("x", [PP, BS * CW], BF16, isOutput=False)
    w1_d = nc.declare_dram_parameter("w1", [C, R], F32, isOutput=False)
    b1_d = nc.declare_dram_parameter("b1", [1, R], F32, isOutput=False)
    w2_d = nc.declare_dram_parameter("w2", [R, C], F32, isOutput=False)
    b2_d = nc.declare_dram_parameter("b2", [1, C], F32, isOutput=False)
    wb_d = nc.declare_dram_parameter("wband", [PP, 10 * PP], BF16,
                                     isOutput=False)
    bc_d = nc.declare_dram_parameter("bconv", [1, 1], F32, isOutput=False)
    out_d = nc.declare_dram_parameter("out", [PP, BS * CW], BF16, isOutput=True)

    with tile.TileContext(nc) as tc:
        with tc.tile_pool(name="const", bufs=1) as cp, \
             tc.tile_pool(name="xbuf", bufs=1) as xp, \
             tc.tile_pool(name="work", bufs=3) as wp, \
             tc.tile_pool(name="sp", bufs=2) as spp, \
             tc.tile_pool(name="psPool", bufs=2, space="PSUM") as psP, \
             tc.tile_pool(name="psMlp", bufs=1, space="PSUM") as psM, \
             tc.tile_pool(name="psB", bufs=2, space="PSUM") as psC:

            # ---------- constants ----------
            identb = cp.tile([PP, PP], BF16)
            masks.make_identity(nc, identb[:])
            ident2 = cp.tile([2, 2], F32)
            masks.make_identity(nc, ident2[:])
            onesb = cp.tile([PP, 1], BF16)
            nc.gpsimd.memset(onesb[:], 1.0)
            ones2 = cp.tile([2, PP], BF16)
            nc.gpsimd.memset(ones2[:], 1.0)

            w1t = cp.tile([128, 2 * R], F32)
            nc.sync.dma_start(w1t[:, 0:R], w1_d[0:128, :])
            nc.sync.dma_start(w1t[:, R:2 * R], w1_d[128:256, :])
            w2t = cp.tile([R, C], F32)
            nc.sync.dma_start(w2t[:], w2_d[:])
            wb = cp.tile([PP, 10 * PP], BF16)
            nc.sync.dma_start(wb[:], wb_d[:])

            b1r = cp.tile([1, R], F32)
            nc.sync.dma_start(b1r[:], b1_d[:])
            b1b = cp.tile([2, R], F32)
            nc.gpsimd.partition_broadcast(b1b[:], b1r[:], channels=2)
            b2r = cp.tile([1, C], F32)
            nc.sync.dma_start(b2r[:], b2_d[:])
            b2b = cp.tile([2, C], F32)
            nc.gpsimd.partition_broadcast(b2b[:], b2r[:], channels=2)
            bcr = cp.tile([1, 1], F32)
            nc.sync.dma_start(bcr[:], bc_d[:])
            bcb = cp.tile([PP, 1], F32)
            nc.gpsimd.partition_broadcast(bcb[:], bcr[:], channels=PP)

            # resident x in packed padded-plane layout (overwritten in place)
            xbuf = xp.tile([PP, BS * CW], BF16)

            def xt(s, t):
                return xbuf[:, (s * NVT + t) * C:(s * NVT + t + 1) * C]

            HCW = (NVT // 2) * C           # first 14 tiles
            for s in range(BS):
                c0 = s * CW
                nc.sync.dma_start(xbuf[:, c0:c0 + HCW], x_d[:, c0:c0 + HCW])
                nc.sync.dma_start(xbuf[:, c0 + HCW:c0 + CW],
                                  x_d[:, c0 + HCW:c0 + CW])

            for s in range(BS):
                # ===== phase A: channel pooling (sum via PE, max via DVE) ====
                accs = [psP.tile([128, 1], F32, tag="poolacc",
                                 name=f"acc{s}_{i}") for i in range(2)]
                for t in range(NVT):
                    v = xt(s, t)
                    for c in range(2):
                        nc.tensor.matmul(accs[c][:], v[:, c * 128:(c + 1) * 128],
                                         onesb[:], start=(t == 0),
                                         stop=(t == NVT - 1),
                                         skip_group_check=True)
                # channel-max over pixels: log-depth slab tree of TT max ops
                # (overlapping slabs are fine -- max is idempotent)
                mt28 = wp.tile([PP, 14 * C], BF16, tag="maxtree")
                base = s * CW

                def xs_(a, b):          # tiles [a, b) of sample s
                    return xbuf[:, base + a * C:base + b * C]

                nc.vector.tensor_tensor(out=mt28[:], in0=xs_(0, 14),
                                        in1=xs_(14, 28), op=AL.max)
                for n, m in ((14, 7), (7, 4), (4, 2), (2, 1)):
                    nc.vector.tensor_tensor(
                        out=mt28[:, 0:m * C], in0=mt28[:, 0:m * C],
                        in1=mt28[:, (n - m) * C:n * C], op=AL.max)
                maxacc = wp.tile([PP, C], BF16, tag="maxacc")
                nc.vector.tensor_tensor(out=maxacc[:], in0=mt28[:, 0:C],
                                        in1=xs_(28, 29), op=AL.max)

                # pooled vectors -> rhs_s [128, 4]: (avg0, max0, avg1, max1)
                rhs_s = wp.tile([128, 4], F32, tag="rhs")
                for c in range(2):
                    nc.scalar.activation(rhs_s[:, 2 * c:2 * c + 1], accs[c][:],
                                         AF.Copy, scale=1.0 / NP)
                    mt = psC.tile([128, PP], BF16, tag="psmT",
                                  name=f"mt{s}_{c}")
                    nc.tensor.transpose(mt[:], maxacc[:, c * 128:(c + 1) * 128],
                                        identb[:])
                    nc.vector.reduce_max(rhs_s[:, 2 * c + 1:2 * c + 2], mt[:],
                                         axis=AX.X)

                # ===== phase B: MLP -> per-channel scale (sum of 2 sigmoids) ==
                mlp_ps = psM.tile([128, 512], F32, tag="psb")
                h_ps = mlp_ps[0:2, 0:R]
                hT_ps = mlp_ps[0:R, R:R + 2]
                co_ps = mlp_ps[0:2, 256:512]
                nc.tensor.matmul(h_ps, rhs_s[:, 0:2], w1t[:, 0:R],
                                 start=True, stop=False, skip_group_check=True)
                nc.tensor.matmul(h_ps, rhs_s[:, 2:4], w1t[:, R:2 * R],
                                 start=False, stop=True, skip_group_check=True)
                hb = wp.tile([2, R], F32, tag="hb")
                nc.vector.tensor_tensor(out=hb[:], in0=h_ps, in1=b1b[:],
                                        op=AL.add)
                hr = wp.tile([2, R], F32, tag="hr")
                nc.scalar.activation(hr[:], hb[:], AF.Relu)
                nc.tensor.transpose(hT_ps, hr[:], ident2[:])
                hT = wp.tile([R, 2], F32, tag="hT")
                nc.vector.tensor_copy(hT[:], hT_ps)
                nc.tensor.matmul(co_ps, hT[:], w2t[:], start=True, stop=True,
                                 skip_group_check=True)
                co_sb = wp.tile([2, C], F32, tag="co")
                nc.vector.tensor_tensor(out=co_sb[:], in0=co_ps, in1=b2b[:],
                                        op=AL.add)
                sig = wp.tile([2, C], BF16, tag="sig")
                nc.scalar.activation(sig[:], co_sb[:], AF.Sigmoid)
                cb_ps = psM.tile([PP, C], F32, tag="pscb")
                nc.tensor.matmul(cb_ps[:], ones2[:], sig[:], start=True,
                                 stop=True, skip_group_check=True)
                cb = wp.tile([PP, C], BF16, tag="cb")
                nc.vector.tensor_copy(cb[:], cb_ps[:])

                # ===== phase C: xg (in place) + spatial mean/max ============
                # single sample-wide DVE ops on 3D access patterns
                v3 = xbuf[:, s * CW:(s + 1) * CW].rearrange(
                    "p (t c) -> p t c", c=C)
                cb3 = cb[:].unsqueeze(1).broadcast_to([PP, NVT, C])
                nc.vector.tensor_tensor(out=v3, in0=v3, in1=cb3, op=AL.mult)
                spm = spp.tile([PP, 36], BF16, tag="spm")
                spx = spp.tile([PP, 36], BF16, tag="spx")
                nc.vector.memset(spm[:], 0.0)
                nc.vector.memset(spx[:], 0.0)
                with nc.allow_low_precision("DVE accumulates fp32 internally; "
                                            "bf16 out feeds 7x7 conv"):
                    nc.vector.reduce_sum(spm[:, 3:32], v3, axis=AX.X)
                nc.vector.reduce_max(spx[:, 3:32], v3, axis=AX.X)

                # ===== phase D: 7x7x2 conv as 10 banded matmuls =============
                conv_ps = psC.tile([PP, NVT], F32, tag="conv",
                                   name=f"conv{s}")
                k = 0
                for ch, sp_t in enumerate((spm, spx)):
                    for j in range(-2, 3):
                        nc.tensor.matmul(
                            conv_ps[:],
                            wb[:, (ch * 5 + j + 2) * PP:(ch * 5 + j + 3) * PP],
                            sp_t[:, 3 + j:32 + j],
                            start=(k == 0), stop=(k == 9),
                            skip_group_check=True)
                        k += 1
                spsc = wp.tile([PP, NVT], F32, tag="spsc")
                nc.scalar.activation(spsc[:], conv_ps[:], AF.Sigmoid,
                                     bias=bcb[:])

                # ===== phase E: out = xg * spatial (in place), store ========
                for t in range(NVT):
                    v = xt(s, t)
                    nc.scalar.activation(v, v, AF.Copy, scale=spsc[:, t:t + 1])
                    if t == NVT // 2 - 1:
                        nc.scalar.dma_start(
                            out_d[:, s * CW:s * CW + HCW],
                            xbuf[:, s * CW:s * CW + HCW])
                nc.scalar.dma_start(out_d[:, s * CW + HCW:(s + 1) * CW],
                                    xbuf[:, s * CW + HCW:(s + 1) * CW])

    nc.compile()
    return nc


def _get_compiled():
    global _COMPILED
    if _COMPILED is None:
        _COMPILED = _build()
    return _COMPILED


def _band_matrices(wconv):
    # Wb[ch, j+2, i, o]: contribution of packed input row i (tile tau+j) to
    # packed output row o (tile tau). Packed row i <-> padded row
    # i + 8*(i>=56); plane offset (dy-3)*PW + (dx-3) in flat 64x64 layout.
    Wb = np.zeros((2, 5, PP, PP), np.float32)
    for ch in range(2):
        wch = wconv[:, :, ch, 0].astype(np.float32)
        if ch == 0:
            wch = wch / C      # fold channel-mean 1/C into the conv weight
        for dy in range(7):
            for dx in range(7):
                off = (dy - 3) * PW + (dx - 3)
                for o in range(PP):
                    r = o + 8 * (o >= 56)        # padded output row
                    t = r + off
                    j = t // 128
                    rpp = t - j * 128            # padded input row
                    if rpp < 56:
                        i = rpp
                    elif 64 <= rpp < 120:
                        i = rpp - 8
                    else:
                        continue                 # lands in a zero pad row
                    Wb[ch, j + 2, i, o] = wch[dy, dx]
    return np.ascontiguousarray(
        Wb.transpose(2, 0, 1, 3).reshape(PP, 10 * PP)).astype(
            ml_dtypes.bfloat16)


def _pack_x(x):
    # [B,H,W,C] f32 -> per-core [PP, BS*CW] bf16 in packed padded-plane layout
    xc = np.asarray(x, np.float32).reshape(N_CORES, BS, H, W, C)
    P = np.zeros((N_CORES, BS, PW, PW, C), np.float32)
    P[:, :, 3:59, 0:56, :] = xc
    T = P.reshape(N_CORES, BS, NTP, 128, C)[:, :, VT0:VT1 + 1]
    Tp = np.concatenate([T[:, :, :, 0:56, :], T[:, :, :, 64:120, :]], axis=3)
    Tp = Tp.transpose(0, 3, 1, 2, 4)             # [NC, PP, BS, NVT, C]
    return np.ascontiguousarray(
        Tp.reshape(N_CORES, PP, BS * CW)).astype(ml_dtypes.bfloat16)


def _unpack_out(res):
    # per-core [PP, BS*CW] bf16 -> [B,H,W,C] f32
    o = np.stack([np.asarray(res[i], np.float32) for i in range(N_CORES)], 0)
    o = o.reshape(N_CORES, PP, BS, NVT, C).transpose(0, 2, 3, 1, 4)
    O = np.zeros((N_CORES, BS, NTP, 128, C), np.float32)
    O[:, :, VT0:VT1 + 1, 0:56] = o[:, :, :, 0:56]
    O[:, :, VT0:VT1 + 1, 64:120] = o[:, :, :, 56:112]
    return O.reshape(N_CORES, BS, PW, PW, C)[:, :, 3:59, 0:56, :].reshape(
        B, H, W, C)


def _make_in_maps(inputs):
    xp = _pack_x(inputs["x"])
    wband = _band_matrices(np.asarray(inputs["wconv"], np.float32))
    w1 = np.asarray(inputs["w1"], np.float32)
    b1 = np.asarray(inputs["b1"], np.float32).reshape(1, R)
    w2 = np.asarray(inputs["w2"], np.float32)
    b2 = np.asarray(inputs["b2"], np.float32).reshape(1, C)
    bc = np.asarray(inputs["bconv"], np.float32).reshape(1, 1)
    return [{
        "x": xp[i], "w1": w1, "b1": b1, "w2": w2, "b2": b2,
        "wband": wband, "bconv": bc,
    } for i in range(N_CORES)]


def kernel(x, w1, b1, w2, b2, wconv, bconv):
    nc = _get_compiled()
    in_maps = _make_in_maps(dict(x=x, w1=w1, b1=b1, w2=w2, b2=b2,
                                 wconv=wconv, bconv=bconv))
    res = run_bass_kernel_spmd(nc, in_maps, list(range(N_CORES)))
    return _unpack_out([res.results[i]["out"] for i in range(N_CORES)])
